# revision 20
# baseline (speedup 1.0000x reference)
"""Tensor-parallel GQA attention prefill (B=1, S=2048, D=4096, 32 q-heads /
8 kv-heads, RoPE, causal) for 8 Trainium2 NeuronCores.

Sharding: head-parallel. Core g owns q-heads 4g..4g+3 and kv-head g
(exact GQA group); host sums the 8 partial output projections.

v2 pipeline (single fused loop over the four 512-row seq blocks b):
  - projections run c-inner over all 6 output tiles (4 q + k + v) with six
    PSUM banks accumulating, xt streamed in 64KB chunks per seq block
    (PE never waits on the 16MB activation load);
  - causality means attention for q-block b needs only blocks <= b of
    K/V, so attention(b) runs right after block b's RoPE;
  - attention jq>=1 runs fp8 (e4m3): exp emits shifted weights
    exp(s*scale - 6.5) straight to fp8 pair tiles, attnV contracts
    kt-pairs with DoubleRow matmuls; jq=0 (diagonal-only) stays bf16;
  - softmax denominators: DVE accumulates exp tiles into f32, one
    bf16 ones-matmul per (block, head) broadcasts them into PSUM;
  - the output projection is emitted as a generator whose matmuls are
    woven between attention ops: out-proj block b-1 fills the PE while
    attention b waits on the scalar engine's exps;
  - PSUM->SBUF evictions for the output projection run on gpsimd, and
    output stores alternate between two DMA queues so the store tail
    never serializes on one queue;
  - all PSUM tiles come from one 8-buffer pool; the live set is kept
    <= 8 banks at every program point.

All matmuls bf16 except attnV/denominator (fp8 DoubleRow); PSUM fp32.
"""

import sys

if "/opt/trn_rl_repo" not in sys.path:
    sys.path.insert(0, "/opt/trn_rl_repo")

from contextlib import ExitStack

import numpy as np
import ml_dtypes

import concourse.bass as bass
import concourse.tile as tile
from concourse import mybir, bacc

BF16 = mybir.dt.bfloat16
F32 = mybir.dt.float32
F16 = mybir.dt.float16
E4 = mybir.dt.float8e4
NBF = ml_dtypes.bfloat16

S = 2048
D = 4096
HD = 128
HQ = 4                      # q heads per core
N_CORES = 8
NCT = D // 128              # 32 contraction tiles over model dim
NSB = S // 512              # 4 seq blocks
NST = S // 128              # 16 seq tiles
NO = HQ + 1                 # rope'd o-tiles: 4 q heads + 1 k head
NOV = NO + 1                # + v head
SCALE = 1.0 / float(np.sqrt(128.0))
NEG = -1e9
CSHIFT = 6.5                # exp shift so fp8 attention weights stay finite

# toggles for A/B testing
FP8_ATTN = True
INTERLEAVE_P3 = True


def build_nc(num_devices=N_CORES):
    nc = bacc.Bacc("TRN2", target_bir_lowering=False, debug=False,
                   num_devices=num_devices)
    xt_d = nc.dram_tensor("xt", [NSB, 128, NCT * 512], BF16, kind="ExternalInput")
    wt_d = nc.dram_tensor("wt", [NOV, 128, NCT * 128], BF16, kind="ExternalInput")
    wot_d = nc.dram_tensor("wot", [HQ, 128, D], BF16, kind="ExternalInput")
    cos2_d = nc.dram_tensor("cos2", [128, S], F16, kind="ExternalInput")
    sin2_d = nc.dram_tensor("sin2", [128, S], F16, kind="ExternalInput")
    jt_d = nc.dram_tensor("jt", [128, 128], BF16, kind="ExternalInput")
    id_d = nc.dram_tensor("ident", [128, 128], BF16, kind="ExternalInput")
    mask_d = nc.dram_tensor("maskt", [128, 128], BF16, kind="ExternalInput")
    out_d = nc.dram_tensor("out", [S, D], BF16, kind="ExternalOutput")

    with tile.TileContext(nc) as tc, ExitStack() as outer:
        # ---------------- persistent SBUF ----------------
        wp = outer.enter_context(tc.tile_pool(name="wall", bufs=1))
        const = outer.enter_context(tc.tile_pool(name="const", bufs=1))
        csp = outer.enter_context(tc.tile_pool(name="cossin", bufs=1))
        qkp = outer.enter_context(tc.tile_pool(name="qkrot", bufs=1))
        vp = outer.enter_context(tc.tile_pool(name="vnat", bufs=1))
        aotp = outer.enter_context(tc.tile_pool(name="aot", bufs=1))
        wotp = outer.enter_context(tc.tile_pool(name="wotsb", bufs=1))

        w_sb = wp.tile([128, NOV * NCT * 128], BF16)     # [p, o*4096 + c*128 + f]
        jt_sb = const.tile([128, 128], BF16)
        id_sb = const.tile([128, 128], BF16)
        mask_sb = const.tile([128, 128], BF16)
        ones_sb = const.tile([128, 128], BF16)
        cos_sb = csp.tile([128, S], F16)
        sin_sb = csp.tile([128, S], F16)
        k_rot = qkp.tile([128, S], BF16)                 # rope'd K, T-layout
        v8 = vp.tile([128, NST, 128], E4)                # [k_local, kt, d] fp8
        nvb = 4 if FP8_ATTN else NST
        v4b = vp.tile([128, nvb, 128], BF16)             # bf16 v (jq0 / fallback)
        aot = aotp.tile([128, HQ * S], BF16)             # attn out, T-layout
        wot_sb = wotp.tile([128, HQ * D], BF16)

        # earliest DMAs: the weights needed by the first matmuls. Two fine
        # rounds (o fastest) so all six o-stationaries for low c arrive
        # first, then one big chunk per o — few triggers, no queue backlog.
        for cb in range(2):
            for o in range(NOV):
                nc.sync.dma_start(
                    out=w_sb[:, o * 4096 + cb * 512: o * 4096 + (cb + 1) * 512],
                    in_=wt_d[o, :, cb * 512:(cb + 1) * 512])
        for o in range(NOV):
            nc.sync.dma_start(
                out=w_sb[:, o * 4096 + 1024:(o + 1) * 4096],
                in_=wt_d[o, :, 1024:])
        nc.sync.dma_start(out=jt_sb, in_=jt_d[:])
        nc.sync.dma_start(out=id_sb, in_=id_d[:])
        nc.sync.dma_start(out=mask_sb, in_=mask_d[:])
        nc.vector.memset(ones_sb, 1.0)
        nbias = const.tile([128, 1], F32)
        nc.vector.memset(nbias, -CSHIFT)
        ones8 = const.tile([128, 2, 128], E4)
        nc.vector.memset(ones8, 1.0)
        nc.sync.dma_start(out=cos_sb, in_=cos2_d[:])
        nc.sync.dma_start(out=sin_sb, in_=sin2_d[:])
        for j in range(HQ):
            for half in range(2):
                hw_ = D // 2
                nc.sync.dma_start(
                    out=wot_sb[:, j * D + half * hw_: j * D + (half + 1) * hw_],
                    in_=wot_d[j, :, half * hw_:(half + 1) * hw_])

        xtp = outer.enter_context(tc.tile_pool(name="xtp", bufs=2))
        qrp = outer.enter_context(tc.tile_pool(name="qrot", bufs=2))

        xbufs = {}

        def load_xt(b, nchunk):
            xb = xtp.tile([128, NCT * 512], BF16, tag="xtb", name=f"xtb_{b}")
            cw = NCT * 512 // nchunk
            for k in range(nchunk):
                nc.scalar.dma_start(out=xb[:, k * cw:(k + 1) * cw],
                                    in_=xt_d[b, :, k * cw:(k + 1) * cw])
            xbufs[b] = xb

        load_xt(0, 8)
        load_xt(1, 2)
        qts = outer.enter_context(tc.tile_pool(name="qtmp", bufs=2))
        vts = outer.enter_context(tc.tile_pool(name="vtsb", bufs=2))
        rtm = outer.enter_context(tc.tile_pool(name="ropetmp", bufs=2))
        etp = outer.enter_context(tc.tile_pool(name="expt", bufs=5))
        rbp = outer.enter_context(tc.tile_pool(name="rbc", bufs=2))
        stg = outer.enter_context(tc.tile_pool(name="stage", bufs=3))

        # single PSUM pool; live tiles are kept <= 8 banks at all times
        psp = outer.enter_context(tc.tile_pool(name="ps8", bufs=8, space="PSUM"))

        # warm the PE p-state during DMA-bound init: dummy matmuls ramp the
        # clock so the first real projections run at full speed
        nc.vector.memset(k_rot[:, 0:512], 0.0)
        wmps = psp.tile([128, 512], F32, tag="ps", name="wmps")
        for _ in range(22):
            nc.tensor.matmul(wmps, k_rot[:, 0:128], k_rot[:, 0:512],
                             start=True, stop=True)

        # ---------------- phase 3 generator ----------------
        # Yields "mm" after each out-proj matmul and "bnd" after a group's
        # evictions (<= 2 PSUM tiles live inside a group, 0 at "bnd").
        EBG = [(0, 2), (2, 4), (4, 6), (6, 8)]

        def p3_block(blk):
            for stc in range(4 * blk, 4 * blk + 4):
                for g0, g1 in EBG:
                    psl = [psp.tile([128, 512], F32, tag="ps",
                                    name=f"p3_{stc}_{eb}")
                           for eb in range(g0, g1)]
                    for j in range(HQ):
                        for i, eb in enumerate(range(g0, g1)):
                            nc.tensor.matmul(
                                psl[i],
                                aot[:, j * S + stc * 128: j * S + (stc + 1) * 128],
                                wot_sb[:, j * D + eb * 512: j * D + eb * 512 + 512],
                                start=(j == 0), stop=(j == HQ - 1))
                            yield "mm"
                    stage = stg.tile([128, 1024], BF16, tag="stage")
                    for i, eb in enumerate(range(g0, g1)):
                        drain = blk == NSB - 1
                        if drain and i % 2 == 1:
                            nc.scalar.activation(
                                out=stage[:, i * 512:(i + 1) * 512], in_=psl[i],
                                func=mybir.ActivationFunctionType.Copy)
                        else:
                            nc.vector.tensor_copy(
                                stage[:, i * 512:(i + 1) * 512], psl[i])
                    eng = nc.scalar if (blk == NSB - 1 and stc % 2 == 1) \
                        else nc.sync
                    eng.dma_start(
                        out=out_d[stc * 128:(stc + 1) * 128,
                                  g0 * 512:g1 * 512],
                        in_=stage)
                    yield "bnd"

        # out-proj stream state: block k may only be pulled once
        # attention-k has fully written aot (allowed >= k).
        p3s = {"blk": 0, "gen": None}

        def p3_fill(n, allowed):
            if not INTERLEAVE_P3 and allowed < NSB - 1:
                return
            got = 0
            while got < n:
                if p3s["gen"] is None:
                    if p3s["blk"] > allowed:
                        return
                    p3s["gen"] = p3_block(p3s["blk"])
                k = next(p3s["gen"], None)
                if k is None:
                    p3s["blk"] += 1
                    p3s["gen"] = None
                elif k == "mm":
                    got += 1

        def p3_boundary():
            # drain to a group boundary so no p3 PSUM tiles stay live
            if p3s["gen"] is None:
                return
            while True:
                k = next(p3s["gen"], None)
                if k is None:
                    p3s["blk"] += 1
                    p3s["gen"] = None
                    return
                if k == "bnd":
                    return

        # ---------------- fused per-block loop ----------------
        ps_store = {}

        def kv1_gen():
            # block 1's K/V projections, woven into block 0's attention as
            # PE filler (block 0 has no out-proj work available yet); evicts
            # its own output so no PSUM tile outlives the generator
            kvo = [4, 5]
            tiles = {o: psp.tile([128, 512], F32, tag="ps", name=f"pj_1_{o}")
                     for o in kvo}
            xb1 = xbufs[1]
            for c in range(NCT):
                for o in kvo:
                    nc.tensor.matmul(
                        tiles[o],
                        w_sb[:, o * 4096 + c * 128: o * 4096 + (c + 1) * 128],
                        xb1[:, c * 512:(c + 1) * 512],
                        start=(c == 0), stop=(c == NCT - 1))
                    yield "mm"
            for o in kvo:
                ps_store[(1, o)] = tiles[o]

        kv1 = {"gen": kv1_gen(), "done": False}

        def kv1_fill(n):
            if kv1["done"]:
                return 0
            got = 0
            while got < n:
                if next(kv1["gen"], None) is None:
                    kv1["done"] = True
                    break
                got += 1
            return got

        def fill(n, b):
            # generic PE filler: out-proj stream, else block-1 K/V proj
            got = 0
            if b == 0 and not kv1["done"]:
                got = kv1_fill(n)
            if got < n:
                p3_fill(n - got, b - 1)

        for b in range(NSB):
            if b + 2 < NSB:
                load_xt(b + 2, 2)
            xb = xbufs.pop(b)
            qrot = qrp.tile([128, HQ * 512], BF16, tag="qr", name=f"qr_{b}")
            kv1_evict = None
            if b == 1:
                while not kv1["done"]:
                    kv1_fill(16)
                # scalar copies now (scalar is idle); the PE-side J-matmul
                # and V transposes are deferred into the c-loop below so
                # they don't block it while the copies run
                qt1 = qts.tile([128, 512], BF16, tag="qt")
                nc.scalar.activation(out=qt1, in_=ps_store.pop((1, 4)),
                                     func=mybir.ActivationFunctionType.Copy)
                vt1 = vts.tile([128, 512], BF16, tag="vt")
                nc.scalar.activation(out=vt1, in_=ps_store.pop((1, 5)),
                                     func=mybir.ActivationFunctionType.Copy)

                def kv1_evict():
                    jp = psp.tile([128, 512], F32, tag="ps", name="jp")
                    nc.tensor.matmul(jp, jt_sb, qt1, start=True, stop=True)
                    t1 = rtm.tile([128, 512], F32, tag="rt")
                    nc.vector.tensor_mul(t1, qt1, cos_sb[:, 512:1024])
                    nc.vector.tensor_mul(jp, jp, sin_sb[:, 512:1024])
                    nc.vector.tensor_add(k_rot[:, 512:1024], t1, jp)
                    for t in range(4):
                        tp = psp.tile([128, 128], BF16, tag="ps", name="vtr")
                        nc.tensor.transpose(
                            tp, vt1[:, t * 128:(t + 1) * 128], id_sb)
                        nc.vector.tensor_copy(v8[:, 4 + t, :], tp)
            done_o = [4, 5] if b == 1 else []
            olist = [o for o in range(NOV) if o not in done_o]
            ps = {o: psp.tile([128, 512], F32, tag="ps", name=f"pj_{b}_{o}")
                  for o in olist}
            for c in range(NCT):
                if c == 8 and kv1_evict is not None:
                    kv1_evict()
                for o in olist:
                    nc.tensor.matmul(
                        ps[o],
                        w_sb[:, o * 4096 + c * 128: o * 4096 + (c + 1) * 128],
                        xb[:, c * 512:(c + 1) * 512],
                        start=(c == 0), stop=(c == NCT - 1))

            # evictions + RoPE; k (o=4) first so attention can begin early.
            # PSUM live: 6 proj tiles shrinking + 1 J tile + <=2 p3 tiles.
            def rope(o):
                qt_sb = qts.tile([128, 512], BF16, tag="qt")
                nc.scalar.activation(out=qt_sb, in_=ps[o],
                                     func=mybir.ActivationFunctionType.Copy)
                jp = psp.tile([128, 512], F32, tag="ps", name="jp")
                nc.tensor.matmul(jp, jt_sb, qt_sb, start=True, stop=True)
                t1 = rtm.tile([128, 512], F32, tag="rt")
                meng = nc.gpsimd if o in (1, 2, 3) else nc.vector
                meng.tensor_mul(t1, qt_sb, cos_sb[:, b * 512:(b + 1) * 512])
                nc.vector.tensor_mul(jp, jp, sin_sb[:, b * 512:(b + 1) * 512])
                dst = (k_rot[:, b * 512:(b + 1) * 512] if o == 4
                       else qrot[:, o * 512:(o + 1) * 512])
                nc.vector.tensor_add(dst, t1, jp)

            def evict_v():
                vt_sb = vts.tile([128, 512], BF16, tag="vt")
                nc.scalar.activation(out=vt_sb, in_=ps[5],
                                     func=mybir.ActivationFunctionType.Copy)
                for t in range(4):
                    kt = 4 * b + t
                    tp = psp.tile([128, 128], BF16, tag="ps", name="vtr")
                    nc.tensor.transpose(tp, vt_sb[:, t * 128:(t + 1) * 128], id_sb)
                    nc.vector.tensor_copy(v8[:, kt, :], tp)
                    if kt < nvb:
                        nc.vector.tensor_copy(v4b[:, kt, :], tp)

            if b != 1:
                rope(4)       # k first
            rope(0)
            rope(1)
            fill(2, b)
            rope(2)
            fill(2, b)
            rope(3)
            fill(2, b)
            if b != 1:
                evict_v()
            fill(6, b)

            # ---------------- attention for q-block b ----------------
            nk = 4 * b + 4
            fp8b = FP8_ATTN and b > 0

            for h in range(HQ):
                outps = psp.tile([128, 512], F32, tag="ps", name="outps")
                den = psp.tile([128, 512], F32, tag="ps", name="den")

                if not fp8b:
                    # bf16 path (block 0: all four k-tiles are diagonal)
                    for kt in range(nk):
                        delta = kt - 4 * b
                        a = max(delta, 0) * 128
                        sps = psp.tile([128, 512], F32, tag="ps", name="sps")
                        nc.tensor.matmul(
                            sps[:, a:],
                            k_rot[:, kt * 128:(kt + 1) * 128],
                            qrot[:, h * 512 + a:(h + 1) * 512],
                            start=True, stop=True)
                        if delta >= 0:
                            nc.vector.tensor_add(
                                sps[:, a:a + 128], sps[:, a:a + 128], mask_sb)
                        et = etp.tile([128, 512], BF16, tag="et", name="etb")
                        nc.scalar.activation(
                            out=et[:, a:], in_=sps[:, a:],
                            func=mybir.ActivationFunctionType.Exp,
                            scale=SCALE, bias=nbias)
                        nc.tensor.matmul(
                            outps[:, a:], v4b[:, kt, :], et[:, a:],
                            start=(kt == 0), stop=(kt == nk - 1))
                        nc.tensor.matmul(
                            den[:, a:], ones_sb, et[:, a:],
                            start=(kt == 0), stop=(kt == nk - 1))
                        fill(2, b)
                else:
                    # fp8 DoubleRow path: kt pairs
                    for p in range(nk // 2):
                        k0, k1 = 2 * p, 2 * p + 1
                        d0, d1 = k0 - 4 * b, k1 - 4 * b
                        a0 = max(d0, 0) * 128
                        a1 = max(d1, 0) * 128
                        et = etp.tile([128, 2, 512], E4, tag="et", name="etp8")
                        for i, (kt, a, dlt) in enumerate(((k0, a0, d0),
                                                          (k1, a1, d1))):
                            sps = psp.tile([128, 512], F32, tag="ps", name="sps")
                            nc.tensor.matmul(
                                sps[:, a:],
                                k_rot[:, kt * 128:(kt + 1) * 128],
                                qrot[:, h * 512 + a:(h + 1) * 512],
                                start=True, stop=True)
                            if dlt >= 0:
                                nc.vector.tensor_add(
                                    sps[:, a:a + 128], sps[:, a:a + 128],
                                    mask_sb)
                            nc.scalar.activation(
                                out=et[:, i, a:], in_=sps[:, a:],
                                func=mybir.ActivationFunctionType.Exp,
                                scale=SCALE, bias=nbias)
                            fill(2, b)
                        if a1 > a0:
                            # dead q-range of the later diagonal tile
                            nc.vector.memset(et[:, 1, a0:a1], 0.0)
                        nc.tensor.matmul(
                            outps[:, a0:],
                            v8[:, k0:k0 + 2, :],
                            et[:, :, a0:],
                            start=(p == 0), stop=(p == nk // 2 - 1),
                            perf_mode=mybir.MatmulPerfMode.DoubleRow)
                        nc.tensor.matmul(
                            den[:, a0:],
                            ones8,
                            et[:, :, a0:],
                            start=(p == 0), stop=(p == nk // 2 - 1),
                            perf_mode=mybir.MatmulPerfMode.DoubleRow)
                        fill(1, b)

                rinv = rbp.tile([128, 512], F32, tag="rinv")
                nc.vector.reciprocal_approx_fast(out=rinv, in_=den)
                nc.vector.tensor_mul(
                    aot[:, h * S + b * 512: h * S + b * 512 + 512],
                    outps, rinv)
                fill(2, b)

            # clear any live p3 PSUM tiles before the next block's proj pass
            p3_boundary()

        # drain remaining out-proj work
        if not INTERLEAVE_P3:
            p3s["blk"], p3s["gen"] = 0, None
        p3_fill(1 << 30, NSB - 1)

    nc.compile()
    return nc


# ---------------------------------------------------------------------------
# host-side prep


def make_consts(cos, sin):
    """cos/sin: [S, 64] f32 -> replicated T-layout + J + identity + diag mask."""
    cos2 = np.repeat(np.ascontiguousarray(cos.T), 2, axis=0).astype(np.float16)
    sin2 = np.repeat(np.ascontiguousarray(sin.T), 2, axis=0).astype(np.float16)
    J = np.zeros((128, 128), np.float32)
    for p in range(64):
        J[2 * p, 2 * p + 1] = -1.0
        J[2 * p + 1, 2 * p] = 1.0
    jt = np.ascontiguousarray(J.T).astype(NBF)
    ident = np.eye(128, dtype=NBF)
    k_idx = np.arange(128)[:, None]
    q_idx = np.arange(128)[None, :]
    maskt = np.where(q_idx >= k_idx, 0.0, NEG).astype(np.float32)  # [k, q]
    return cos2, sin2, jt, ident, maskt.astype(NBF)


def prep_all(x, wq, wk, wv, wo, cos, sin, n_cores=N_CORES):
    x2 = np.asarray(x, np.float32).reshape(S, D)
    xt = np.ascontiguousarray(x2.T).astype(NBF)          # [D, S] = [(c,p),(b,col)]
    xt = np.ascontiguousarray(
        xt.reshape(NCT, 128, NSB, 512).transpose(2, 1, 0, 3)
    ).reshape(NSB, 128, NCT * 512)
    wq = np.asarray(wq, np.float32)
    wk = np.asarray(wk, np.float32)
    wv = np.asarray(wv, np.float32)
    wo = np.asarray(wo, np.float32)
    cos2, sin2, jt, ident, maskt = make_consts(
        np.asarray(cos, np.float32), np.asarray(sin, np.float32))
    in_maps = []
    for g in range(n_cores):
        w_cat = np.concatenate(
            [wq[g * 512:(g + 1) * 512],
             wk[g * 128:(g + 1) * 128],
             wv[g * 128:(g + 1) * 128]], axis=0)          # [768, D]
        # wt[o, p, c*128 + f] = w_cat[o*128 + f, c*128 + p]
        wt = np.ascontiguousarray(
            w_cat.reshape(6, 128, NCT, 128).transpose(0, 3, 2, 1)
        ).reshape(6, 128, NCT * 128).astype(NBF)
        wot = np.ascontiguousarray(
            wo[:, g * 512:(g + 1) * 512].T).reshape(4, 128, D).astype(NBF)
        in_maps.append({
            "xt": xt, "wt": wt, "wot": wot, "cos2": cos2, "sin2": sin2,
            "jt": jt, "ident": ident, "maskt": maskt,
        })
    return in_maps


_NC_CACHE = None


def _get_nc():
    global _NC_CACHE
    if _NC_CACHE is None:
        _NC_CACHE = build_nc()
    return _NC_CACHE


def kernel(x, wq, wk, wv, wo, cos, sin, mask, start_pos):
    # mask is the standard causal mask (start_pos=0 prefill) — the kernel
    # applies causality structurally, so neither input is shipped.
    from concourse.bass_utils import run_bass_kernel_spmd

    nc = _get_nc()
    in_maps = prep_all(x, wq, wk, wv, wo, cos, sin)
    res = run_bass_kernel_spmd(nc, in_maps, core_ids=list(range(N_CORES)))
    acc = np.zeros((S, D), np.float32)
    for r in res.results:
        acc += r["out"].astype(np.float32)
    return acc.reshape(1, S, D)


# revision 21
# speedup vs baseline: 1.0440x; 1.0440x over previous
"""Tensor-parallel GQA attention prefill (B=1, S=2048, D=4096, 32 q-heads /
8 kv-heads, RoPE, causal) for 8 Trainium2 NeuronCores.

Sharding: head-parallel. Core g owns q-heads 4g..4g+3 and kv-head g
(exact GQA group); host sums the 8 partial output projections.

v2 pipeline (single fused loop over the four 512-row seq blocks b):
  - projections run c-inner over all 6 output tiles (4 q + k + v) with six
    PSUM banks accumulating, xt streamed in 64KB chunks per seq block
    (PE never waits on the 16MB activation load);
  - causality means attention for q-block b needs only blocks <= b of
    K/V, so attention(b) runs right after block b's RoPE;
  - attention jq>=1 runs fp8 (e4m3): exp emits shifted weights
    exp(s*scale - 6.5) straight to fp8 pair tiles, attnV contracts
    kt-pairs with DoubleRow matmuls; jq=0 (diagonal-only) stays bf16;
  - softmax denominators: DVE accumulates exp tiles into f32, one
    bf16 ones-matmul per (block, head) broadcasts them into PSUM;
  - the output projection is emitted as a generator whose matmuls are
    woven between attention ops: out-proj block b-1 fills the PE while
    attention b waits on the scalar engine's exps;
  - PSUM->SBUF evictions for the output projection run on gpsimd, and
    output stores alternate between two DMA queues so the store tail
    never serializes on one queue;
  - all PSUM tiles come from one 8-buffer pool; the live set is kept
    <= 8 banks at every program point.

All matmuls bf16 except attnV/denominator (fp8 DoubleRow); PSUM fp32.
"""

import sys

if "/opt/trn_rl_repo" not in sys.path:
    sys.path.insert(0, "/opt/trn_rl_repo")

from contextlib import ExitStack

import numpy as np
import ml_dtypes

import concourse.bass as bass
import concourse.tile as tile
from concourse import mybir, bacc

BF16 = mybir.dt.bfloat16
F32 = mybir.dt.float32
F16 = mybir.dt.float16
E4 = mybir.dt.float8e4
NBF = ml_dtypes.bfloat16

S = 2048
D = 4096
HD = 128
HQ = 4                      # q heads per core
N_CORES = 8
NCT = D // 128              # 32 contraction tiles over model dim
NSB = S // 512              # 4 seq blocks
NST = S // 128              # 16 seq tiles
NO = HQ + 1                 # rope'd o-tiles: 4 q heads + 1 k head
NOV = NO + 1                # + v head
SCALE = 1.0 / float(np.sqrt(128.0))
NEG = -1e9
CSHIFT = 6.5                # exp shift so fp8 attention weights stay finite

# toggles for A/B testing
FP8_ATTN = True
INTERLEAVE_P3 = True


def build_nc(num_devices=N_CORES):
    nc = bacc.Bacc("TRN2", target_bir_lowering=False, debug=False,
                   num_devices=num_devices)
    xt_d = nc.dram_tensor("xt", [NSB, 128, NCT * 512], BF16, kind="ExternalInput")
    wt_d = nc.dram_tensor("wt", [NOV, 128, NCT * 128], BF16, kind="ExternalInput")
    wot_d = nc.dram_tensor("wot", [HQ, 128, D], BF16, kind="ExternalInput")
    cos2_d = nc.dram_tensor("cos2", [128, S], F16, kind="ExternalInput")
    sin2_d = nc.dram_tensor("sin2", [128, S], F16, kind="ExternalInput")
    jt_d = nc.dram_tensor("jt", [128, 128], BF16, kind="ExternalInput")
    id_d = nc.dram_tensor("ident", [128, 128], BF16, kind="ExternalInput")
    mask_d = nc.dram_tensor("maskt", [128, 128], BF16, kind="ExternalInput")
    out_d = nc.dram_tensor("out", [S, D], BF16, kind="ExternalOutput")

    with tile.TileContext(nc) as tc, ExitStack() as outer:
        # ---------------- persistent SBUF ----------------
        wp = outer.enter_context(tc.tile_pool(name="wall", bufs=1))
        const = outer.enter_context(tc.tile_pool(name="const", bufs=1))
        csp = outer.enter_context(tc.tile_pool(name="cossin", bufs=1))
        qkp = outer.enter_context(tc.tile_pool(name="qkrot", bufs=1))
        vp = outer.enter_context(tc.tile_pool(name="vnat", bufs=1))
        aotp = outer.enter_context(tc.tile_pool(name="aot", bufs=1))
        wotp = outer.enter_context(tc.tile_pool(name="wotsb", bufs=1))

        w_sb = wp.tile([128, NOV * NCT * 128], BF16)     # [p, o*4096 + c*128 + f]
        jt_sb = const.tile([128, 128], BF16)
        id_sb = const.tile([128, 128], BF16)
        mask_sb = const.tile([128, 128], BF16)
        ones_sb = const.tile([128, 128], BF16)
        cos_sb = csp.tile([128, S], F16)
        sin_sb = csp.tile([128, S], F16)
        k_rot = qkp.tile([128, S], BF16)                 # rope'd K, T-layout
        v8 = vp.tile([128, NST, 128], E4)                # [k_local, kt, d] fp8
        nvb = 4 if FP8_ATTN else NST
        v4b = vp.tile([128, nvb, 128], BF16)             # bf16 v (jq0 / fallback)
        aot = aotp.tile([128, HQ * S], BF16)             # attn out, T-layout
        wot_sb = wotp.tile([128, HQ * D], BF16)

        # earliest DMAs: weights in rounds matched to the c-loop's
        # consumption order — each round delivers all six o-stationaries
        # for the next c-tile span before the PE reaches it.
        for cb in range(2):
            for o in range(NOV):
                nc.sync.dma_start(
                    out=w_sb[:, o * 4096 + cb * 512: o * 4096 + (cb + 1) * 512],
                    in_=wt_d[o, :, cb * 512:(cb + 1) * 512])
        for rnd in range(3):
            lo, hi = 1024 + rnd * 1024, 2048 + rnd * 1024
            for o in range(NOV):
                nc.sync.dma_start(
                    out=w_sb[:, o * 4096 + lo: o * 4096 + hi],
                    in_=wt_d[o, :, lo:hi])
        nc.sync.dma_start(out=jt_sb, in_=jt_d[:])
        nc.sync.dma_start(out=id_sb, in_=id_d[:])
        nc.sync.dma_start(out=mask_sb, in_=mask_d[:])
        nc.vector.memset(ones_sb, 1.0)
        nbias = const.tile([128, 1], F32)
        nc.vector.memset(nbias, -CSHIFT)
        ones8 = const.tile([128, 2, 128], E4)
        nc.vector.memset(ones8, 1.0)
        nc.sync.dma_start(out=cos_sb, in_=cos2_d[:])
        nc.sync.dma_start(out=sin_sb, in_=sin2_d[:])
        for j in range(HQ):
            for half in range(2):
                hw_ = D // 2
                nc.sync.dma_start(
                    out=wot_sb[:, j * D + half * hw_: j * D + (half + 1) * hw_],
                    in_=wot_d[j, :, half * hw_:(half + 1) * hw_])

        xtp = outer.enter_context(tc.tile_pool(name="xtp", bufs=2))
        qrp = outer.enter_context(tc.tile_pool(name="qrot", bufs=2))

        xbufs = {}

        def load_xt(b, nchunk):
            xb = xtp.tile([128, NCT * 512], BF16, tag="xtb", name=f"xtb_{b}")
            cw = NCT * 512 // nchunk
            for k in range(nchunk):
                nc.scalar.dma_start(out=xb[:, k * cw:(k + 1) * cw],
                                    in_=xt_d[b, :, k * cw:(k + 1) * cw])
            xbufs[b] = xb

        load_xt(0, 8)
        load_xt(1, 2)
        qts = outer.enter_context(tc.tile_pool(name="qtmp", bufs=2))
        vts = outer.enter_context(tc.tile_pool(name="vtsb", bufs=2))
        rtm = outer.enter_context(tc.tile_pool(name="ropetmp", bufs=2))
        etp = outer.enter_context(tc.tile_pool(name="expt", bufs=5))
        rbp = outer.enter_context(tc.tile_pool(name="rbc", bufs=2))
        stg = outer.enter_context(tc.tile_pool(name="stage", bufs=3))

        # single PSUM pool; live tiles are kept <= 8 banks at all times
        psp = outer.enter_context(tc.tile_pool(name="ps8", bufs=8, space="PSUM"))

        # warm the PE p-state during DMA-bound init: dummy matmuls ramp the
        # clock so the first real projections run at full speed
        nc.vector.memset(k_rot[:, 0:512], 0.0)
        wmps = psp.tile([128, 512], F32, tag="ps", name="wmps")
        for _ in range(22):
            nc.tensor.matmul(wmps, k_rot[:, 0:128], k_rot[:, 0:512],
                             start=True, stop=True)

        # ---------------- phase 3 generator ----------------
        # Yields "mm" after each out-proj matmul and "bnd" after a group's
        # evictions (<= 2 PSUM tiles live inside a group, 0 at "bnd").
        EBG = [(0, 2), (2, 4), (4, 6), (6, 8)]

        def p3_block(blk):
            for stc in range(4 * blk, 4 * blk + 4):
                for g0, g1 in EBG:
                    psl = [psp.tile([128, 512], F32, tag="ps",
                                    name=f"p3_{stc}_{eb}")
                           for eb in range(g0, g1)]
                    for j in range(HQ):
                        for i, eb in enumerate(range(g0, g1)):
                            nc.tensor.matmul(
                                psl[i],
                                aot[:, j * S + stc * 128: j * S + (stc + 1) * 128],
                                wot_sb[:, j * D + eb * 512: j * D + eb * 512 + 512],
                                start=(j == 0), stop=(j == HQ - 1))
                            yield "mm"
                    stage = stg.tile([128, 1024], BF16, tag="stage")
                    for i, eb in enumerate(range(g0, g1)):
                        drain = blk == NSB - 1
                        if drain and i % 2 == 1:
                            nc.scalar.activation(
                                out=stage[:, i * 512:(i + 1) * 512], in_=psl[i],
                                func=mybir.ActivationFunctionType.Copy)
                        else:
                            nc.vector.tensor_copy(
                                stage[:, i * 512:(i + 1) * 512], psl[i])
                    eng = nc.scalar if (blk == NSB - 1 and stc % 2 == 1) \
                        else nc.sync
                    eng.dma_start(
                        out=out_d[stc * 128:(stc + 1) * 128,
                                  g0 * 512:g1 * 512],
                        in_=stage)
                    yield "bnd"

        # out-proj stream state: block k may only be pulled once
        # attention-k has fully written aot (allowed >= k).
        p3s = {"blk": 0, "gen": None}

        def p3_fill(n, allowed):
            if not INTERLEAVE_P3 and allowed < NSB - 1:
                return
            got = 0
            while got < n:
                if p3s["gen"] is None:
                    if p3s["blk"] > allowed:
                        return
                    p3s["gen"] = p3_block(p3s["blk"])
                k = next(p3s["gen"], None)
                if k is None:
                    p3s["blk"] += 1
                    p3s["gen"] = None
                elif k == "mm":
                    got += 1

        def p3_boundary():
            # drain to a group boundary so no p3 PSUM tiles stay live
            if p3s["gen"] is None:
                return
            while True:
                k = next(p3s["gen"], None)
                if k is None:
                    p3s["blk"] += 1
                    p3s["gen"] = None
                    return
                if k == "bnd":
                    return

        # ---------------- fused per-block loop ----------------
        ps_store = {}

        def kv1_gen():
            # block 1's K/V projections, woven into block 0's attention as
            # PE filler (block 0 has no out-proj work available yet); evicts
            # its own output so no PSUM tile outlives the generator
            kvo = [4, 5]
            tiles = {o: psp.tile([128, 512], F32, tag="ps", name=f"pj_1_{o}")
                     for o in kvo}
            xb1 = xbufs[1]
            for c in range(NCT):
                for o in kvo:
                    nc.tensor.matmul(
                        tiles[o],
                        w_sb[:, o * 4096 + c * 128: o * 4096 + (c + 1) * 128],
                        xb1[:, c * 512:(c + 1) * 512],
                        start=(c == 0), stop=(c == NCT - 1))
                    yield "mm"
            for o in kvo:
                ps_store[(1, o)] = tiles[o]

        kv1 = {"gen": kv1_gen(), "done": False}

        def kv1_fill(n):
            if kv1["done"]:
                return 0
            got = 0
            while got < n:
                if next(kv1["gen"], None) is None:
                    kv1["done"] = True
                    break
                got += 1
            return got

        def fill(n, b):
            # generic PE filler: out-proj stream, else block-1 K/V proj
            got = 0
            if b == 0 and not kv1["done"]:
                got = kv1_fill(n)
            if got < n:
                p3_fill(n - got, b - 1)

        for b in range(NSB):
            if b + 2 < NSB:
                load_xt(b + 2, 2)
            xb = xbufs.pop(b)
            qrot = qrp.tile([128, HQ * 512], BF16, tag="qr", name=f"qr_{b}")
            kv1_evict = None
            if b == 1:
                while not kv1["done"]:
                    kv1_fill(16)
                # scalar copies now (scalar is idle); the PE-side J-matmul
                # and V transposes are deferred into the c-loop below so
                # they don't block it while the copies run
                qt1 = qts.tile([128, 512], BF16, tag="qt")
                nc.scalar.activation(out=qt1, in_=ps_store.pop((1, 4)),
                                     func=mybir.ActivationFunctionType.Copy)
                vt1 = vts.tile([128, 512], BF16, tag="vt")
                nc.scalar.activation(out=vt1, in_=ps_store.pop((1, 5)),
                                     func=mybir.ActivationFunctionType.Copy)

                def kv1_evict():
                    jp = psp.tile([128, 512], F32, tag="ps", name="jp")
                    nc.tensor.matmul(jp, jt_sb, qt1, start=True, stop=True)
                    t1 = rtm.tile([128, 512], F32, tag="rt")
                    nc.vector.tensor_mul(t1, qt1, cos_sb[:, 512:1024])
                    nc.vector.tensor_mul(jp, jp, sin_sb[:, 512:1024])
                    nc.vector.tensor_add(k_rot[:, 512:1024], t1, jp)
                    for t in range(4):
                        tp = psp.tile([128, 128], BF16, tag="ps", name="vtr")
                        nc.tensor.transpose(
                            tp, vt1[:, t * 128:(t + 1) * 128], id_sb)
                        nc.vector.tensor_copy(v8[:, 4 + t, :], tp)
            done_o = [4, 5] if b == 1 else []
            olist = [o for o in range(NOV) if o not in done_o]
            ps = {o: psp.tile([128, 512], F32, tag="ps", name=f"pj_{b}_{o}")
                  for o in olist}
            for c in range(NCT):
                if c == 8 and kv1_evict is not None:
                    kv1_evict()
                for o in olist:
                    nc.tensor.matmul(
                        ps[o],
                        w_sb[:, o * 4096 + c * 128: o * 4096 + (c + 1) * 128],
                        xb[:, c * 512:(c + 1) * 512],
                        start=(c == 0), stop=(c == NCT - 1))

            # evictions + RoPE; k (o=4) first so attention can begin early.
            # PSUM live: 6 proj tiles shrinking + 1 J tile + <=2 p3 tiles.
            def rope(o):
                qt_sb = qts.tile([128, 512], BF16, tag="qt")
                nc.scalar.activation(out=qt_sb, in_=ps[o],
                                     func=mybir.ActivationFunctionType.Copy)
                jp = psp.tile([128, 512], F32, tag="ps", name="jp")
                nc.tensor.matmul(jp, jt_sb, qt_sb, start=True, stop=True)
                t1 = rtm.tile([128, 512], F32, tag="rt")
                meng = nc.gpsimd if o in (1, 2, 3) else nc.vector
                meng.tensor_mul(t1, qt_sb, cos_sb[:, b * 512:(b + 1) * 512])
                nc.vector.tensor_mul(jp, jp, sin_sb[:, b * 512:(b + 1) * 512])
                dst = (k_rot[:, b * 512:(b + 1) * 512] if o == 4
                       else qrot[:, o * 512:(o + 1) * 512])
                nc.vector.tensor_add(dst, t1, jp)

            def evict_v():
                vt_sb = vts.tile([128, 512], BF16, tag="vt")
                nc.scalar.activation(out=vt_sb, in_=ps[5],
                                     func=mybir.ActivationFunctionType.Copy)
                for t in range(4):
                    kt = 4 * b + t
                    tp = psp.tile([128, 128], BF16, tag="ps", name="vtr")
                    nc.tensor.transpose(tp, vt_sb[:, t * 128:(t + 1) * 128], id_sb)
                    nc.vector.tensor_copy(v8[:, kt, :], tp)
                    if kt < nvb:
                        nc.vector.tensor_copy(v4b[:, kt, :], tp)

            if b != 1:
                rope(4)       # k first
            rope(0)
            rope(1)
            fill(2, b)
            rope(2)
            fill(2, b)
            rope(3)
            fill(2, b)
            if b != 1:
                evict_v()
            fill(6, b)

            # ---------------- attention for q-block b ----------------
            nk = 4 * b + 4
            fp8b = FP8_ATTN and b > 0

            for h in range(HQ):
                outps = psp.tile([128, 512], F32, tag="ps", name="outps")
                den = psp.tile([128, 512], F32, tag="ps", name="den")

                if not fp8b:
                    # bf16 path (block 0: all four k-tiles are diagonal)
                    for kt in range(nk):
                        delta = kt - 4 * b
                        a = max(delta, 0) * 128
                        sps = psp.tile([128, 512], F32, tag="ps", name="sps")
                        nc.tensor.matmul(
                            sps[:, a:],
                            k_rot[:, kt * 128:(kt + 1) * 128],
                            qrot[:, h * 512 + a:(h + 1) * 512],
                            start=True, stop=True)
                        if delta >= 0:
                            nc.vector.tensor_add(
                                sps[:, a:a + 128], sps[:, a:a + 128], mask_sb)
                        et = etp.tile([128, 512], BF16, tag="et", name="etb")
                        nc.scalar.activation(
                            out=et[:, a:], in_=sps[:, a:],
                            func=mybir.ActivationFunctionType.Exp,
                            scale=SCALE, bias=nbias)
                        nc.tensor.matmul(
                            outps[:, a:], v4b[:, kt, :], et[:, a:],
                            start=(kt == 0), stop=(kt == nk - 1))
                        nc.tensor.matmul(
                            den[:, a:], ones_sb, et[:, a:],
                            start=(kt == 0), stop=(kt == nk - 1))
                        fill(2, b)
                else:
                    # fp8 DoubleRow path: kt pairs
                    for p in range(nk // 2):
                        k0, k1 = 2 * p, 2 * p + 1
                        d0, d1 = k0 - 4 * b, k1 - 4 * b
                        a0 = max(d0, 0) * 128
                        a1 = max(d1, 0) * 128
                        et = etp.tile([128, 2, 512], E4, tag="et", name="etp8")
                        for i, (kt, a, dlt) in enumerate(((k0, a0, d0),
                                                          (k1, a1, d1))):
                            sps = psp.tile([128, 512], F32, tag="ps", name="sps")
                            nc.tensor.matmul(
                                sps[:, a:],
                                k_rot[:, kt * 128:(kt + 1) * 128],
                                qrot[:, h * 512 + a:(h + 1) * 512],
                                start=True, stop=True)
                            if dlt >= 0:
                                nc.vector.tensor_add(
                                    sps[:, a:a + 128], sps[:, a:a + 128],
                                    mask_sb)
                            nc.scalar.activation(
                                out=et[:, i, a:], in_=sps[:, a:],
                                func=mybir.ActivationFunctionType.Exp,
                                scale=SCALE, bias=nbias)
                            fill(2, b)
                        if a1 > a0:
                            # dead q-range of the later diagonal tile
                            nc.vector.memset(et[:, 1, a0:a1], 0.0)
                        nc.tensor.matmul(
                            outps[:, a0:],
                            v8[:, k0:k0 + 2, :],
                            et[:, :, a0:],
                            start=(p == 0), stop=(p == nk // 2 - 1),
                            perf_mode=mybir.MatmulPerfMode.DoubleRow)
                        nc.tensor.matmul(
                            den[:, a0:],
                            ones8,
                            et[:, :, a0:],
                            start=(p == 0), stop=(p == nk // 2 - 1),
                            perf_mode=mybir.MatmulPerfMode.DoubleRow)
                        fill(1, b)

                rinv = rbp.tile([128, 512], F32, tag="rinv")
                nc.vector.reciprocal_approx_fast(out=rinv, in_=den)
                nc.vector.tensor_mul(
                    aot[:, h * S + b * 512: h * S + b * 512 + 512],
                    outps, rinv)
                fill(2, b)

            # clear any live p3 PSUM tiles before the next block's proj pass
            p3_boundary()

        # drain remaining out-proj work
        if not INTERLEAVE_P3:
            p3s["blk"], p3s["gen"] = 0, None
        p3_fill(1 << 30, NSB - 1)

    nc.compile()
    return nc


# ---------------------------------------------------------------------------
# host-side prep


def make_consts(cos, sin):
    """cos/sin: [S, 64] f32 -> replicated T-layout + J + identity + diag mask."""
    cos2 = np.repeat(np.ascontiguousarray(cos.T), 2, axis=0).astype(np.float16)
    sin2 = np.repeat(np.ascontiguousarray(sin.T), 2, axis=0).astype(np.float16)
    J = np.zeros((128, 128), np.float32)
    for p in range(64):
        J[2 * p, 2 * p + 1] = -1.0
        J[2 * p + 1, 2 * p] = 1.0
    jt = np.ascontiguousarray(J.T).astype(NBF)
    ident = np.eye(128, dtype=NBF)
    k_idx = np.arange(128)[:, None]
    q_idx = np.arange(128)[None, :]
    maskt = np.where(q_idx >= k_idx, 0.0, NEG).astype(np.float32)  # [k, q]
    return cos2, sin2, jt, ident, maskt.astype(NBF)


def prep_all(x, wq, wk, wv, wo, cos, sin, n_cores=N_CORES):
    x2 = np.asarray(x, np.float32).reshape(S, D)
    xt = np.ascontiguousarray(x2.T).astype(NBF)          # [D, S] = [(c,p),(b,col)]
    xt = np.ascontiguousarray(
        xt.reshape(NCT, 128, NSB, 512).transpose(2, 1, 0, 3)
    ).reshape(NSB, 128, NCT * 512)
    wq = np.asarray(wq, np.float32)
    wk = np.asarray(wk, np.float32)
    wv = np.asarray(wv, np.float32)
    wo = np.asarray(wo, np.float32)
    cos2, sin2, jt, ident, maskt = make_consts(
        np.asarray(cos, np.float32), np.asarray(sin, np.float32))
    in_maps = []
    for g in range(n_cores):
        w_cat = np.concatenate(
            [wq[g * 512:(g + 1) * 512],
             wk[g * 128:(g + 1) * 128],
             wv[g * 128:(g + 1) * 128]], axis=0)          # [768, D]
        # wt[o, p, c*128 + f] = w_cat[o*128 + f, c*128 + p]
        wt = np.ascontiguousarray(
            w_cat.reshape(6, 128, NCT, 128).transpose(0, 3, 2, 1)
        ).reshape(6, 128, NCT * 128).astype(NBF)
        wot = np.ascontiguousarray(
            wo[:, g * 512:(g + 1) * 512].T).reshape(4, 128, D).astype(NBF)
        in_maps.append({
            "xt": xt, "wt": wt, "wot": wot, "cos2": cos2, "sin2": sin2,
            "jt": jt, "ident": ident, "maskt": maskt,
        })
    return in_maps


_NC_CACHE = None


def _get_nc():
    global _NC_CACHE
    if _NC_CACHE is None:
        _NC_CACHE = build_nc()
    return _NC_CACHE


def kernel(x, wq, wk, wv, wo, cos, sin, mask, start_pos):
    # mask is the standard causal mask (start_pos=0 prefill) — the kernel
    # applies causality structurally, so neither input is shipped.
    from concourse.bass_utils import run_bass_kernel_spmd

    nc = _get_nc()
    in_maps = prep_all(x, wq, wk, wv, wo, cos, sin)
    res = run_bass_kernel_spmd(nc, in_maps, core_ids=list(range(N_CORES)))
    acc = np.zeros((S, D), np.float32)
    for r in res.results:
        acc += r["out"].astype(np.float32)
    return acc.reshape(1, S, D)


# revision 22
# speedup vs baseline: 1.0446x; 1.0006x over previous
"""Tensor-parallel GQA attention prefill (B=1, S=2048, D=4096, 32 q-heads /
8 kv-heads, RoPE, causal) for 8 Trainium2 NeuronCores.

Sharding: head-parallel. Core g owns q-heads 4g..4g+3 and kv-head g
(exact GQA group); host sums the 8 partial output projections.

v2 pipeline (single fused loop over the four 512-row seq blocks b):
  - projections run c-inner over all 6 output tiles (4 q + k + v) with six
    PSUM banks accumulating, xt streamed in 64KB chunks per seq block
    (PE never waits on the 16MB activation load);
  - causality means attention for q-block b needs only blocks <= b of
    K/V, so attention(b) runs right after block b's RoPE;
  - attention jq>=1 runs fp8 (e4m3): exp emits shifted weights
    exp(s*scale - 6.5) straight to fp8 pair tiles, attnV contracts
    kt-pairs with DoubleRow matmuls; jq=0 (diagonal-only) stays bf16;
  - softmax denominators: DVE accumulates exp tiles into f32, one
    bf16 ones-matmul per (block, head) broadcasts them into PSUM;
  - the output projection is emitted as a generator whose matmuls are
    woven between attention ops: out-proj block b-1 fills the PE while
    attention b waits on the scalar engine's exps;
  - PSUM->SBUF evictions for the output projection run on gpsimd, and
    output stores alternate between two DMA queues so the store tail
    never serializes on one queue;
  - all PSUM tiles come from one 8-buffer pool; the live set is kept
    <= 8 banks at every program point.

All matmuls bf16 except attnV/denominator (fp8 DoubleRow); PSUM fp32.
"""

import sys

if "/opt/trn_rl_repo" not in sys.path:
    sys.path.insert(0, "/opt/trn_rl_repo")

from contextlib import ExitStack

import numpy as np
import ml_dtypes

import concourse.bass as bass
import concourse.tile as tile
from concourse import mybir, bacc

BF16 = mybir.dt.bfloat16
F32 = mybir.dt.float32
F16 = mybir.dt.float16
E4 = mybir.dt.float8e4
NBF = ml_dtypes.bfloat16

S = 2048
D = 4096
HD = 128
HQ = 4                      # q heads per core
N_CORES = 8
NCT = D // 128              # 32 contraction tiles over model dim
NSB = S // 512              # 4 seq blocks
NST = S // 128              # 16 seq tiles
NO = HQ + 1                 # rope'd o-tiles: 4 q heads + 1 k head
NOV = NO + 1                # + v head
SCALE = 1.0 / float(np.sqrt(128.0))
NEG = -1e9
CSHIFT = 6.5                # exp shift so fp8 attention weights stay finite

# toggles for A/B testing
FP8_ATTN = True
INTERLEAVE_P3 = True


def build_nc(num_devices=N_CORES):
    nc = bacc.Bacc("TRN2", target_bir_lowering=False, debug=False,
                   num_devices=num_devices)
    xt_d = nc.dram_tensor("xt", [NSB, 128, NCT * 512], BF16, kind="ExternalInput")
    wt_d = nc.dram_tensor("wt", [NOV, 128, NCT * 128], BF16, kind="ExternalInput")
    wot_d = nc.dram_tensor("wot", [HQ, 128, D], BF16, kind="ExternalInput")
    cos2_d = nc.dram_tensor("cos2", [128, S], F16, kind="ExternalInput")
    sin2_d = nc.dram_tensor("sin2", [128, S], F16, kind="ExternalInput")
    jt_d = nc.dram_tensor("jt", [128, 128], BF16, kind="ExternalInput")
    id_d = nc.dram_tensor("ident", [128, 128], BF16, kind="ExternalInput")
    mask_d = nc.dram_tensor("maskt", [128, 128], BF16, kind="ExternalInput")
    out_d = nc.dram_tensor("out", [S, D], BF16, kind="ExternalOutput")

    with tile.TileContext(nc) as tc, ExitStack() as outer:
        # ---------------- persistent SBUF ----------------
        wp = outer.enter_context(tc.tile_pool(name="wall", bufs=1))
        const = outer.enter_context(tc.tile_pool(name="const", bufs=1))
        csp = outer.enter_context(tc.tile_pool(name="cossin", bufs=1))
        qkp = outer.enter_context(tc.tile_pool(name="qkrot", bufs=1))
        vp = outer.enter_context(tc.tile_pool(name="vnat", bufs=1))
        aotp = outer.enter_context(tc.tile_pool(name="aot", bufs=1))
        wotp = outer.enter_context(tc.tile_pool(name="wotsb", bufs=1))

        w_sb = wp.tile([128, NOV * NCT * 128], BF16)     # [p, o*4096 + c*128 + f]
        jt_sb = const.tile([128, 128], BF16)
        id_sb = const.tile([128, 128], BF16)
        mask_sb = const.tile([128, 128], BF16)
        ones_sb = const.tile([128, 128], BF16)
        cos_sb = csp.tile([128, S], F16)
        sin_sb = csp.tile([128, S], F16)
        k_rot = qkp.tile([128, S], BF16)                 # rope'd K, T-layout
        v8 = vp.tile([128, NST, 128], E4)                # [k_local, kt, d] fp8
        nvb = 4 if FP8_ATTN else NST
        v4b = vp.tile([128, nvb, 128], BF16)             # bf16 v (jq0 / fallback)
        aot = aotp.tile([128, HQ * S], BF16)             # attn out, T-layout
        wot_sb = wotp.tile([128, HQ * D], BF16)

        # earliest DMAs: weights in rounds matched to the c-loop's
        # consumption order — each round delivers all six o-stationaries
        # for the next c-tile span before the PE reaches it.
        for cb in range(2):
            for o in range(NOV):
                nc.sync.dma_start(
                    out=w_sb[:, o * 4096 + cb * 512: o * 4096 + (cb + 1) * 512],
                    in_=wt_d[o, :, cb * 512:(cb + 1) * 512])
        for rnd in range(3):
            lo, hi = 1024 + rnd * 1024, 2048 + rnd * 1024
            for o in range(NOV):
                nc.sync.dma_start(
                    out=w_sb[:, o * 4096 + lo: o * 4096 + hi],
                    in_=wt_d[o, :, lo:hi])
        nc.sync.dma_start(out=jt_sb, in_=jt_d[:])
        nc.sync.dma_start(out=id_sb, in_=id_d[:])
        nc.sync.dma_start(out=mask_sb, in_=mask_d[:])
        nc.vector.memset(ones_sb, 1.0)
        nbias = const.tile([128, 1], F32)
        nc.vector.memset(nbias, -CSHIFT)
        ones8 = const.tile([128, 2, 128], E4)
        nc.vector.memset(ones8, 1.0)
        nc.sync.dma_start(out=cos_sb, in_=cos2_d[:])
        nc.sync.dma_start(out=sin_sb, in_=sin2_d[:])
        for j in range(HQ):
            for half in range(2):
                hw_ = D // 2
                nc.sync.dma_start(
                    out=wot_sb[:, j * D + half * hw_: j * D + (half + 1) * hw_],
                    in_=wot_d[j, :, half * hw_:(half + 1) * hw_])

        xtp = outer.enter_context(tc.tile_pool(name="xtp", bufs=2))
        qrp = outer.enter_context(tc.tile_pool(name="qrot", bufs=2))

        xbufs = {}

        def load_xt(b, nchunk):
            xb = xtp.tile([128, NCT * 512], BF16, tag="xtb", name=f"xtb_{b}")
            cw = NCT * 512 // nchunk
            for k in range(nchunk):
                nc.scalar.dma_start(out=xb[:, k * cw:(k + 1) * cw],
                                    in_=xt_d[b, :, k * cw:(k + 1) * cw])
            xbufs[b] = xb

        load_xt(0, 8)
        load_xt(1, 2)
        qts = outer.enter_context(tc.tile_pool(name="qtmp", bufs=2))
        vts = outer.enter_context(tc.tile_pool(name="vtsb", bufs=2))
        rtm = outer.enter_context(tc.tile_pool(name="ropetmp", bufs=2))
        etp = outer.enter_context(tc.tile_pool(name="expt", bufs=5))
        rbp = outer.enter_context(tc.tile_pool(name="rbc", bufs=2))
        stg = outer.enter_context(tc.tile_pool(name="stage", bufs=3))

        # single PSUM pool; live tiles are kept <= 8 banks at all times
        psp = outer.enter_context(tc.tile_pool(name="ps8", bufs=8, space="PSUM"))

        # warm the PE p-state during DMA-bound init: dummy matmuls ramp the
        # clock so the first real projections run at full speed
        nc.vector.memset(k_rot[:, 0:512], 0.0)
        wmps = psp.tile([128, 512], F32, tag="ps", name="wmps")
        for _ in range(22):
            nc.tensor.matmul(wmps, k_rot[:, 0:128], k_rot[:, 0:512],
                             start=True, stop=True)

        # ---------------- phase 3 generator ----------------
        # Yields "mm" after each out-proj matmul and "bnd" after a group's
        # evictions (<= 2 PSUM tiles live inside a group, 0 at "bnd").
        EBG = [(0, 2), (2, 4), (4, 6), (6, 8)]

        def p3_block(blk):
            for stc in range(4 * blk, 4 * blk + 4):
                for g0, g1 in EBG:
                    psl = [psp.tile([128, 512], F32, tag="ps",
                                    name=f"p3_{stc}_{eb}")
                           for eb in range(g0, g1)]
                    for j in range(HQ):
                        for i, eb in enumerate(range(g0, g1)):
                            nc.tensor.matmul(
                                psl[i],
                                aot[:, j * S + stc * 128: j * S + (stc + 1) * 128],
                                wot_sb[:, j * D + eb * 512: j * D + eb * 512 + 512],
                                start=(j == 0), stop=(j == HQ - 1))
                            yield "mm"
                    stage = stg.tile([128, 1024], BF16, tag="stage")
                    for i, eb in enumerate(range(g0, g1)):
                        drain = blk == NSB - 1
                        if drain and i % 2 == 1:
                            nc.scalar.activation(
                                out=stage[:, i * 512:(i + 1) * 512], in_=psl[i],
                                func=mybir.ActivationFunctionType.Copy)
                        else:
                            nc.vector.tensor_copy(
                                stage[:, i * 512:(i + 1) * 512], psl[i])
                    eng = nc.scalar if (blk == NSB - 1 and stc % 2 == 1) \
                        else nc.sync
                    eng.dma_start(
                        out=out_d[stc * 128:(stc + 1) * 128,
                                  g0 * 512:g1 * 512],
                        in_=stage)
                    yield "bnd"

        # out-proj stream state: block k may only be pulled once
        # attention-k has fully written aot (allowed >= k).
        p3s = {"blk": 0, "gen": None}

        def p3_fill(n, allowed):
            if not INTERLEAVE_P3 and allowed < NSB - 1:
                return
            got = 0
            while got < n:
                if p3s["gen"] is None:
                    if p3s["blk"] > allowed:
                        return
                    p3s["gen"] = p3_block(p3s["blk"])
                k = next(p3s["gen"], None)
                if k is None:
                    p3s["blk"] += 1
                    p3s["gen"] = None
                elif k == "mm":
                    got += 1

        def p3_boundary():
            # drain to a group boundary so no p3 PSUM tiles stay live
            if p3s["gen"] is None:
                return
            while True:
                k = next(p3s["gen"], None)
                if k is None:
                    p3s["blk"] += 1
                    p3s["gen"] = None
                    return
                if k == "bnd":
                    return

        # ---------------- fused per-block loop ----------------
        ps_store = {}

        def kv1_gen():
            # block 1's K/V projections, woven into block 0's attention as
            # PE filler (block 0 has no out-proj work available yet); evicts
            # its own output so no PSUM tile outlives the generator
            kvo = [4, 5]
            tiles = {o: psp.tile([128, 512], F32, tag="ps", name=f"pj_1_{o}")
                     for o in kvo}
            xb1 = xbufs[1]
            for c in range(NCT):
                for o in kvo:
                    nc.tensor.matmul(
                        tiles[o],
                        w_sb[:, o * 4096 + c * 128: o * 4096 + (c + 1) * 128],
                        xb1[:, c * 512:(c + 1) * 512],
                        start=(c == 0), stop=(c == NCT - 1))
                    yield "mm"
            for o in kvo:
                ps_store[(1, o)] = tiles[o]

        kv1 = {"gen": kv1_gen(), "done": False}

        def kv1_fill(n):
            if kv1["done"]:
                return 0
            got = 0
            while got < n:
                if next(kv1["gen"], None) is None:
                    kv1["done"] = True
                    break
                got += 1
            return got

        def fill(n, b):
            # generic PE filler: out-proj stream, else block-1 K/V proj
            got = 0
            if b == 0 and not kv1["done"]:
                got = kv1_fill(n)
            if got < n:
                p3_fill(n - got, b - 1)

        for b in range(NSB):
            if b + 2 < NSB:
                load_xt(b + 2, 2)
            xb = xbufs.pop(b)
            qrot = qrp.tile([128, HQ * 512], BF16, tag="qr", name=f"qr_{b}")
            kv1_evict = None
            if b == 1:
                while not kv1["done"]:
                    kv1_fill(16)
                # scalar copies now (scalar is idle); the PE-side J-matmul
                # and V transposes are deferred into the c-loop below so
                # they don't block it while the copies run
                qt1 = qts.tile([128, 512], BF16, tag="qt")
                nc.scalar.activation(out=qt1, in_=ps_store.pop((1, 4)),
                                     func=mybir.ActivationFunctionType.Copy)
                vt1 = vts.tile([128, 512], BF16, tag="vt")
                nc.scalar.activation(out=vt1, in_=ps_store.pop((1, 5)),
                                     func=mybir.ActivationFunctionType.Copy)

                def kv1_evict():
                    jp = psp.tile([128, 512], F32, tag="ps", name="jp")
                    nc.tensor.matmul(jp, jt_sb, qt1, start=True, stop=True)
                    t1 = rtm.tile([128, 512], F32, tag="rt")
                    nc.vector.tensor_mul(t1, qt1, cos_sb[:, 512:1024])
                    nc.vector.tensor_mul(jp, jp, sin_sb[:, 512:1024])
                    nc.vector.tensor_add(k_rot[:, 512:1024], t1, jp)
                    for t in range(4):
                        tp = psp.tile([128, 128], BF16, tag="ps", name="vtr")
                        nc.tensor.transpose(
                            tp, vt1[:, t * 128:(t + 1) * 128], id_sb)
                        nc.vector.tensor_copy(v8[:, 4 + t, :], tp)
            done_o = [4, 5] if b == 1 else []
            olist = [o for o in range(NOV) if o not in done_o]
            ps = {o: psp.tile([128, 512], F32, tag="ps", name=f"pj_{b}_{o}")
                  for o in olist}
            for c in range(NCT):
                if c == 8 and kv1_evict is not None:
                    kv1_evict()
                for o in olist:
                    nc.tensor.matmul(
                        ps[o],
                        w_sb[:, o * 4096 + c * 128: o * 4096 + (c + 1) * 128],
                        xb[:, c * 512:(c + 1) * 512],
                        start=(c == 0), stop=(c == NCT - 1))

            # evictions + RoPE; k (o=4) first so attention can begin early.
            # PSUM live: 6 proj tiles shrinking + 1 J tile + <=2 p3 tiles.
            def rope(o):
                qt_sb = qts.tile([128, 512], BF16, tag="qt")
                if o in (1, 3):
                    nc.vector.tensor_copy(qt_sb, ps[o])
                else:
                    nc.scalar.activation(out=qt_sb, in_=ps[o],
                                         func=mybir.ActivationFunctionType.Copy)
                jp = psp.tile([128, 512], F32, tag="ps", name="jp")
                nc.tensor.matmul(jp, jt_sb, qt_sb, start=True, stop=True)
                t1 = rtm.tile([128, 512], F32, tag="rt")
                meng = nc.gpsimd if o in (1, 2, 3) else nc.vector
                meng.tensor_mul(t1, qt_sb, cos_sb[:, b * 512:(b + 1) * 512])
                nc.vector.tensor_mul(jp, jp, sin_sb[:, b * 512:(b + 1) * 512])
                dst = (k_rot[:, b * 512:(b + 1) * 512] if o == 4
                       else qrot[:, o * 512:(o + 1) * 512])
                nc.vector.tensor_add(dst, t1, jp)

            def evict_v():
                vt_sb = vts.tile([128, 512], BF16, tag="vt")
                nc.scalar.activation(out=vt_sb, in_=ps[5],
                                     func=mybir.ActivationFunctionType.Copy)
                for t in range(4):
                    kt = 4 * b + t
                    tp = psp.tile([128, 128], BF16, tag="ps", name="vtr")
                    nc.tensor.transpose(tp, vt_sb[:, t * 128:(t + 1) * 128], id_sb)
                    nc.vector.tensor_copy(v8[:, kt, :], tp)
                    if kt < nvb:
                        nc.vector.tensor_copy(v4b[:, kt, :], tp)

            if b != 1:
                rope(4)       # k first
            rope(0)
            rope(1)
            fill(2, b)
            rope(2)
            fill(2, b)
            rope(3)
            fill(2, b)
            if b != 1:
                evict_v()
            fill(6, b)

            # ---------------- attention for q-block b ----------------
            nk = 4 * b + 4
            fp8b = FP8_ATTN and b > 0

            for h in range(HQ):
                outps = psp.tile([128, 512], F32, tag="ps", name="outps")
                den = psp.tile([128, 512], F32, tag="ps", name="den")

                if not fp8b:
                    # bf16 path (block 0: all four k-tiles are diagonal)
                    for kt in range(nk):
                        delta = kt - 4 * b
                        a = max(delta, 0) * 128
                        sps = psp.tile([128, 512], F32, tag="ps", name="sps")
                        nc.tensor.matmul(
                            sps[:, a:],
                            k_rot[:, kt * 128:(kt + 1) * 128],
                            qrot[:, h * 512 + a:(h + 1) * 512],
                            start=True, stop=True)
                        if delta >= 0:
                            nc.vector.tensor_add(
                                sps[:, a:a + 128], sps[:, a:a + 128], mask_sb)
                        et = etp.tile([128, 512], BF16, tag="et", name="etb")
                        nc.scalar.activation(
                            out=et[:, a:], in_=sps[:, a:],
                            func=mybir.ActivationFunctionType.Exp,
                            scale=SCALE, bias=nbias)
                        nc.tensor.matmul(
                            outps[:, a:], v4b[:, kt, :], et[:, a:],
                            start=(kt == 0), stop=(kt == nk - 1))
                        nc.tensor.matmul(
                            den[:, a:], ones_sb, et[:, a:],
                            start=(kt == 0), stop=(kt == nk - 1))
                        fill(2, b)
                else:
                    # fp8 DoubleRow path: kt pairs
                    for p in range(nk // 2):
                        k0, k1 = 2 * p, 2 * p + 1
                        d0, d1 = k0 - 4 * b, k1 - 4 * b
                        a0 = max(d0, 0) * 128
                        a1 = max(d1, 0) * 128
                        et = etp.tile([128, 2, 512], E4, tag="et", name="etp8")
                        for i, (kt, a, dlt) in enumerate(((k0, a0, d0),
                                                          (k1, a1, d1))):
                            sps = psp.tile([128, 512], F32, tag="ps", name="sps")
                            nc.tensor.matmul(
                                sps[:, a:],
                                k_rot[:, kt * 128:(kt + 1) * 128],
                                qrot[:, h * 512 + a:(h + 1) * 512],
                                start=True, stop=True)
                            if dlt >= 0:
                                nc.vector.tensor_add(
                                    sps[:, a:a + 128], sps[:, a:a + 128],
                                    mask_sb)
                            nc.scalar.activation(
                                out=et[:, i, a:], in_=sps[:, a:],
                                func=mybir.ActivationFunctionType.Exp,
                                scale=SCALE, bias=nbias)
                            fill(2, b)
                        if a1 > a0:
                            # dead q-range of the later diagonal tile
                            nc.vector.memset(et[:, 1, a0:a1], 0.0)
                        nc.tensor.matmul(
                            outps[:, a0:],
                            v8[:, k0:k0 + 2, :],
                            et[:, :, a0:],
                            start=(p == 0), stop=(p == nk // 2 - 1),
                            perf_mode=mybir.MatmulPerfMode.DoubleRow)
                        nc.tensor.matmul(
                            den[:, a0:],
                            ones8,
                            et[:, :, a0:],
                            start=(p == 0), stop=(p == nk // 2 - 1),
                            perf_mode=mybir.MatmulPerfMode.DoubleRow)
                        fill(1, b)

                rinv = rbp.tile([128, 512], F32, tag="rinv")
                nc.vector.reciprocal_approx_fast(out=rinv, in_=den)
                nc.vector.tensor_mul(
                    aot[:, h * S + b * 512: h * S + b * 512 + 512],
                    outps, rinv)
                fill(2, b)

            # clear any live p3 PSUM tiles before the next block's proj pass
            p3_boundary()

        # drain remaining out-proj work
        if not INTERLEAVE_P3:
            p3s["blk"], p3s["gen"] = 0, None
        p3_fill(1 << 30, NSB - 1)

    nc.compile()
    return nc


# ---------------------------------------------------------------------------
# host-side prep


def make_consts(cos, sin):
    """cos/sin: [S, 64] f32 -> replicated T-layout + J + identity + diag mask."""
    cos2 = np.repeat(np.ascontiguousarray(cos.T), 2, axis=0).astype(np.float16)
    sin2 = np.repeat(np.ascontiguousarray(sin.T), 2, axis=0).astype(np.float16)
    J = np.zeros((128, 128), np.float32)
    for p in range(64):
        J[2 * p, 2 * p + 1] = -1.0
        J[2 * p + 1, 2 * p] = 1.0
    jt = np.ascontiguousarray(J.T).astype(NBF)
    ident = np.eye(128, dtype=NBF)
    k_idx = np.arange(128)[:, None]
    q_idx = np.arange(128)[None, :]
    maskt = np.where(q_idx >= k_idx, 0.0, NEG).astype(np.float32)  # [k, q]
    return cos2, sin2, jt, ident, maskt.astype(NBF)


def prep_all(x, wq, wk, wv, wo, cos, sin, n_cores=N_CORES):
    x2 = np.asarray(x, np.float32).reshape(S, D)
    xt = np.ascontiguousarray(x2.T).astype(NBF)          # [D, S] = [(c,p),(b,col)]
    xt = np.ascontiguousarray(
        xt.reshape(NCT, 128, NSB, 512).transpose(2, 1, 0, 3)
    ).reshape(NSB, 128, NCT * 512)
    wq = np.asarray(wq, np.float32)
    wk = np.asarray(wk, np.float32)
    wv = np.asarray(wv, np.float32)
    wo = np.asarray(wo, np.float32)
    cos2, sin2, jt, ident, maskt = make_consts(
        np.asarray(cos, np.float32), np.asarray(sin, np.float32))
    in_maps = []
    for g in range(n_cores):
        w_cat = np.concatenate(
            [wq[g * 512:(g + 1) * 512],
             wk[g * 128:(g + 1) * 128],
             wv[g * 128:(g + 1) * 128]], axis=0)          # [768, D]
        # wt[o, p, c*128 + f] = w_cat[o*128 + f, c*128 + p]
        wt = np.ascontiguousarray(
            w_cat.reshape(6, 128, NCT, 128).transpose(0, 3, 2, 1)
        ).reshape(6, 128, NCT * 128).astype(NBF)
        wot = np.ascontiguousarray(
            wo[:, g * 512:(g + 1) * 512].T).reshape(4, 128, D).astype(NBF)
        in_maps.append({
            "xt": xt, "wt": wt, "wot": wot, "cos2": cos2, "sin2": sin2,
            "jt": jt, "ident": ident, "maskt": maskt,
        })
    return in_maps


_NC_CACHE = None


def _get_nc():
    global _NC_CACHE
    if _NC_CACHE is None:
        _NC_CACHE = build_nc()
    return _NC_CACHE


def kernel(x, wq, wk, wv, wo, cos, sin, mask, start_pos):
    # mask is the standard causal mask (start_pos=0 prefill) — the kernel
    # applies causality structurally, so neither input is shipped.
    from concourse.bass_utils import run_bass_kernel_spmd

    nc = _get_nc()
    in_maps = prep_all(x, wq, wk, wv, wo, cos, sin)
    res = run_bass_kernel_spmd(nc, in_maps, core_ids=list(range(N_CORES)))
    acc = np.zeros((S, D), np.float32)
    for r in res.results:
        acc += r["out"].astype(np.float32)
    return acc.reshape(1, S, D)
